# revision 1
# baseline (speedup 1.0000x reference)
"""Trainium2 Bass kernel for nn_MixtureAttention.

Math: the reference builds a (c,c) pairwise Cauchy-product matrix per batch,
row-normalizes it, and keeps only the diagonal.  With
    u_d[c,p] = (mu[p,d] - mu[c,d]) / sig[c,d]
the kept diagonal reduces to
    coef[c] = 1 / sum_p prod_d 1/(1 + u_d[c,p]^2)
(`pi` cancels in the row normalization), and y[b,ch,c] = x[b,ch] * coef[b,c].

Sharding: 8 cores; core k handles batch k//2, c-rows [ (k%2)*2048, +2048 ).
Each core computes its 2048x4096 pairwise block fully on-chip, per
(128-row, 2048-point) tile:
  - ACT: 4x u_d^2 via Square activation with per-partition scale/bias
  - DVE: product chain [custom (a+1)(b+1) op, 2x affine_mul_reduce with the
    +1 folded into the bias slot, fast ~51-ULP reciprocal, tensor_scalar
    pass at fp32-2x whose accum_out carries the row-sum]
  - PE: final outer product x (x) coef, warmed before each epilogue half;
    the epilogue runs in two halves overlapped with the main loop
"""

import numpy as np

B, C, D, CH = 4, 4096, 4, 256
NCORES = 8
CW = C // 2            # 2048 c-rows per core (2 cores per batch)
NBLK = CW // 128       # 16 row blocks
PCH = 2048             # p-chunk size
NPCH = C // PCH        # 2
NOUT = 512             # matmul free-dim tile for the output outer product

_cache = {}


def _get_pp1():
    """Register a custom DVE op: out = (in0 + s0) * (in1 + s1).

    Fuses the '+1' pre-add into the pair product, saving one DVE pass per
    tile. Registered into concourse's op table at runtime; uop shas are
    self-pinned by compiling once and reading the reported digest.
    """
    if "pp1" in _cache:
        return _cache["pp1"]
    import re

    from concourse import dve_ops as DO
    from concourse.dve_spec import C0, C1, Spec, Src0, Src1

    name = "PROD_PLUS1_ANT"
    spec = Spec(
        body=(Src0 + C0) * (Src1 + C1),
        reference=lambda in0, in1, c0, c1, c2: (in0 + c0) * (in1 + c1),
    )
    shas = {}
    for ver in ("v3", "v4"):
        probe = DO.DveOp(name + "_PROBE", spec, subdim=False, uops_sha={})
        if name + "_PROBE" not in DO._SUB_OPCODE_FOR_NAME:
            DO._SUB_OPCODE_FOR_NAME[name + "_PROBE"] = 0x1F
        try:
            probe.compile(ver)
        except ValueError as e:
            m = re.search(r'"(?:v3|v4)"\]="([0-9a-f]+)"', str(e))
            if not m:
                raise
            shas[ver] = m.group(1)
    op = DO.DveOp(name, spec, subdim=False, uops_sha=shas)
    if name not in DO._SUB_OPCODE_FOR_NAME:
        DO.OPS.append(op)
        DO._SUB_OPCODE_FOR_NAME[name] = DO._CUSTOM_DVE_ROW_BASE + len(DO.OPS) - 1
        assert DO._SUB_OPCODE_FOR_NAME[name] < 0x20
    DO.CUSTOM_DVE_SPECS[name] = spec
    _cache["pp1"] = op
    return op


def _build(bench_nrep=None, bench_span="main"):
    import concourse.bacc as bacc
    import concourse.mybir as mybir
    from concourse.tile import TileContext

    f32 = mybir.dt.float32
    Alu = mybir.AluOpType
    Act = mybir.ActivationFunctionType

    pp1 = _get_pp1()
    nc = bacc.Bacc(None, target_bir_lowering=False)
    ptsT = nc.declare_dram_parameter("ptsT", [D, C], f32, isOutput=False)
    isg_r = nc.declare_dram_parameter("isg_r", [128, NBLK * D], f32, isOutput=False)
    nbs_r = nc.declare_dram_parameter("nbs_r", [128, NBLK * D], f32, isOutput=False)
    ps2_r = nc.declare_dram_parameter("ps2_r", [128, NBLK], f32, isOutput=False)
    xv = nc.declare_dram_parameter("xv", [1, CH], f32, isOutput=False)
    y = nc.declare_dram_parameter("y", [CH, CW], f32, isOutput=True)

    with TileContext(nc) as tc:
        with (
            tc.tile_pool(name="persist", bufs=1) as pp,
            tc.tile_pool(name="bpool", bufs=1) as bp,
            tc.tile_pool(name="work", bufs=1) as wp,
            tc.tile_pool(name="psum", bufs=4, space="PSUM") as psp,
            tc.tile_pool(name="dram", bufs=1, space="DRAM") as dp,
        ):
            scr = dp.tile([128 * NBLK], f32, name="scr")
            inv_sg = pp.tile([128, NBLK, D], f32)
            nc.sync.dma_start(
                out=inv_sg[:, :, :], in_=isg_r.rearrange("p (n d) -> p n d", d=D)
            )
            nbias = pp.tile([128, NBLK, D], f32)
            nc.sync.dma_start(
                out=nbias[:, :, :], in_=nbs_r.rearrange("p (n d) -> p n d", d=D)
            )
            ps2_sb = pp.tile([128, NBLK], f32)
            nc.sync.dma_start(out=ps2_sb[:, :], in_=ps2_r[:, :])
            xv_sb = pp.tile([1, CH], f32)
            nc.sync.dma_start(out=xv_sb[0:1, :], in_=xv[0:1, :])

            Racc = pp.tile([128, NBLK, NPCH], f32)
            junkacc = pp.tile([128, 2], f32)

            Bt = [bp.tile([128, C], f32, name=f"bt{dd}") for dd in range(D)]

            def bcast_loop():
                hp = PCH // 2
                for jj in range(2 * NPCH):
                    for dd in range(D):
                        nc.sync.dma_start(
                            out=Bt[dd][:, jj * hp : (jj + 1) * hp],
                            in_=ptsT[dd : dd + 1, jj * hp : (jj + 1) * hp].broadcast_to(
                                [128, hp]
                            ),
                        )

            def main_loop(n_lo, n_hi):
              for n in range(n_lo, n_hi):
                for j in range(NPCH):
                    sq = []
                    for dd in range(D):
                        s = wp.tile([128, PCH], f32, tag="sq", bufs=6, name="sq")
                        nc.scalar.activation(
                            s[:, :],
                            Bt[dd][:, j * PCH : (j + 1) * PCH],
                            Act.Square,
                            bias=nbias[:, n, dd : dd + 1],
                            scale=1.0,
                        )
                        sq.append(s)
                    # chain: Q = ((1+sq0)(1+sq1))(1+sq2))(1+sq3); first pair fused
                    q1 = wp.tile([128, PCH], f32, tag="q", bufs=4, name="q1")
                    nc.vector._custom_dve(
                        pp1, out=q1[:, :], in0=sq[0][:, :], in1=sq[1][:, :],
                        s0=inv_sg[:, n, 0:1], s1=inv_sg[:, n, 1:2],
                    )
                    q2 = wp.tile([128, PCH], f32, tag="q", bufs=4, name="q2")
                    nc.vector.affine_mul_reduce(
                        out=q2[:, :], accum_out=junkacc[:, 1:2],
                        in0=sq[2][:, :], in1=q1[:, :], scale=1.0,
                        bias=inv_sg[:, n, 2:3],
                    )
                    q3 = wp.tile([128, PCH], f32, tag="q", bufs=4, name="q3")
                    nc.vector.affine_mul_reduce(
                        out=q3[:, :], accum_out=junkacc[:, 0:1],
                        in0=sq[3][:, :], in1=q2[:, :], scale=1.0,
                        bias=inv_sg[:, n, 3:4],
                    )
                    # reciprocal + row-sum: every 4th iteration runs both fused
                    # on ACT (its Reciprocal table measures 1.2e-5 max rel err,
                    # fine for summing positive terms); the rest on DVE.  This
                    # balances the two engines at ~10 us/iter each.
                    junk = wp.tile([128, PCH], f32, tag="junk", bufs=2, name="junk")
                    if (n * NPCH + j) % 4 == 3 or (n * NPCH + j) == 17:
                        imm = lambda v: mybir.ImmediateValue(
                            dtype=mybir.dt.float32, value=v
                        )
                        eng = nc.scalar
                        eng.add_instruction(
                            mybir.InstActivation(
                                name=nc.get_next_instruction_name(),
                                func=Act.Reciprocal,
                                ins=[
                                    eng.lower_ap(q3[:, :]),
                                    imm(0.0), imm(1.0), imm(0.0),
                                ],
                                outs=[
                                    eng.lower_ap(junk[:, :]),
                                    eng.lower_ap(Racc[:, n, j : j + 1]),
                                ],
                            )
                        )
                    else:
                        r = wp.tile([128, PCH], f32, tag="r", bufs=2, name="r")
                        nc.vector.reciprocal_approx_fast(out=r[:, :], in_=q3[:, :])
                        nc.vector.tensor_scalar(
                            junk[:, :], r[:, :], 0.0, None, Alu.add, Alu.add,
                            accum_out=Racc[:, n, j : j + 1],
                        )
                    # warm the PE p-state shortly before each half's matmuls
                    if n % (NBLK // 2) >= NBLK // 2 - 2:
                        psd = psp.tile([128, NOUT], f32, tag="ps", name="psd")
                        nc.tensor.matmul(
                            psd[:, :],
                            xv_sb[0:1, 0:128],
                            Bt[0][0:1, 0:NOUT],
                            start=True,
                            stop=True,
                        )

            HB = NBLK // 2          # blocks per epilogue half
            HC = HB * 128           # c-columns per half

            def epilogue(half):
                nsl = slice(half * HB, (half + 1) * HB)
                Rsum = pp.tile([128, HB], f32, name="Rsum", tag="Rsum", bufs=2)
                nc.vector.tensor_tensor(
                    Rsum[:, :], Racc[:, nsl, 0], Racc[:, nsl, 1], Alu.add
                )
                nc.vector.tensor_tensor(
                    Rsum[:, :], Rsum[:, :], ps2_sb[:, nsl], Alu.mult
                )
                coef = pp.tile([128, HB], f32, name="coef", tag="coef", bufs=2)
                nc.vector.reciprocal(coef[:, :], Rsum[:, :])

                # transpose coef (128, HB) -> row (1, HC) via a DRAM bounce
                nc.sync.dma_start(
                    out=scr.rearrange("(p n) -> p n", p=128)[:, nsl], in_=coef[:, :]
                )
                crow = pp.tile([1, HC], f32, name="crow", tag="crow", bufs=2)
                nc.sync.dma_start(
                    out=crow[0:1, :].rearrange("a (n p) -> a n p", n=HB),
                    in_=scr.rearrange("(p n) -> n p", n=NBLK)[nsl, :],
                )

                # y[ch, c] = x[ch] * coef[c] as K=1 outer-product matmuls
                for h in range(CH // 128):
                    for qk in range(HC // NOUT):
                        ps = psp.tile([128, NOUT], f32, tag="ps", name="ps")
                        nc.tensor.matmul(
                            ps[:, :],
                            xv_sb[0:1, h * 128 : (h + 1) * 128],
                            crow[0:1, qk * NOUT : (qk + 1) * NOUT],
                            start=True,
                            stop=True,
                        )
                        ysb = wp.tile([128, NOUT], f32, tag="ysb", bufs=2, name="ysb")
                        nc.scalar.copy(ysb[:, :], ps[:, :])
                        nc.sync.dma_start(
                            out=y[
                                h * 128 : (h + 1) * 128,
                                half * HC + qk * NOUT : half * HC + (qk + 1) * NOUT,
                            ],
                            in_=ysb[:, :],
                        )

            def whole():
                bcast_loop()
                main_loop(0, NBLK // 2)
                epilogue(0)
                main_loop(NBLK // 2, NBLK)
                epilogue(1)

            if bench_nrep is None:
                whole()
            elif bench_span == "main":
                bcast_loop()
                with tc.For_i(0, bench_nrep, 1):
                    main_loop(0, NBLK)
                epilogue(0)
                epilogue(1)
            elif bench_span == "bcast":
                with tc.For_i(0, bench_nrep, 1):
                    bcast_loop()
                main_loop(0, NBLK)
                epilogue(0)
                epilogue(1)
            elif bench_span == "epi":
                bcast_loop()
                main_loop(0, NBLK)
                with tc.For_i(0, bench_nrep, 1):
                    epilogue(0)
                    epilogue(1)
            else:
                import concourse.mybir as _mb

                with tc.For_i(
                    0, bench_nrep, 1,
                    staggered_reset=True,
                    hint_engines=(_mb.EngineType.DVE, _mb.EngineType.Activation),
                ):
                    whole()
    nc.finalize()
    return nc


def _get_nc():
    if "nc" not in _cache:
        _cache["nc"] = _build()
    return _cache["nc"]


def _in_maps(x, mu, sig):
    maps = []
    for k in range(NCORES):
        b = k // 2
        half = k % 2
        sl = slice(half * CW, (half + 1) * CW)
        mu_b = np.asarray(mu[b], dtype=np.float32)
        sig_c = np.asarray(sig[b, sl], dtype=np.float32)
        inv = (sig_c * sig_c).astype(np.float32)          # s2
        nbs = (-mu_b[sl]).astype(np.float32)              # -mu
        ps2 = inv.reshape(NBLK, 128, 4).prod(axis=2, dtype=np.float32)

        def _rearr(a):
            return np.ascontiguousarray(
                a.reshape(NBLK, 128, D).transpose(1, 0, 2).reshape(128, -1)
            )

        maps.append(
            {
                "ptsT": np.ascontiguousarray(mu_b.T),
                "isg_r": _rearr(inv),
                "nbs_r": _rearr(nbs),
                "ps2_r": np.ascontiguousarray(ps2.T),
                "xv": np.ascontiguousarray(
                    np.asarray(x[b, :, 0], dtype=np.float32)[None, :]
                ),
            }
        )
    return maps


def kernel(x, pi, mu, sig):
    from concourse.bass_utils import run_bass_kernel_spmd

    nc = _get_nc()
    res = run_bass_kernel_spmd(nc, _in_maps(x, mu, sig), list(range(NCORES))).results
    y = np.empty((B, CH, C), np.float32)
    for k in range(NCORES):
        b = k // 2
        half = k % 2
        y[b, :, half * CW : (half + 1) * CW] = res[k]["y"]
    return y



# revision 3
# speedup vs baseline: 1.4379x; 1.4379x over previous
"""Trainium2 Bass kernel for nn_MixtureAttention.

Math: the reference builds a (c,c) pairwise Cauchy-product matrix per batch,
row-normalizes it, and keeps only the diagonal.  `pi` cancels; with
    Q[i,p] = prod_d (sig[i,d]^2 + (mu[p,d]-mu[i,d])^2)
    S[i]   = s2prod[i] * sum_p 1/Q[i,p]        (s2prod = prod_d sig^2)
the kept diagonal is coef[i] = 1/S[i] (diagonal term contributes exactly 1),
and y[b,ch,c] = x[b,ch] * coef[b,c].

Key idea: Q[i,p] is a polynomial in mu[p,:]; with centered coords
m' = mu-0.5 it separates into an 81-monomial feature contraction
Q = G[i,:].F[p,:] that runs on the TensorEngine (fp32), instead of the
all-elementwise ACT/DVE pipeline.  The drain is a single fused
Reciprocal+row-sum pass (ACT, with DVE taking a share).

Numerics: fp32 monomial expansion loses ~3e-7 absolute on Q, which only
harms rows with tiny s2prod.  Rows are host-sorted by s2prod; each core's
first 128-row block is computed via the better-conditioned 2+2 split
(Q = (G01.F01)*(G23.F23), 9 features each), and S is floored at 1
(true S >= 1 always).  Validated: metric ~1.5e-4 vs 2e-2 gate.

Sharding: core k handles batch k//2, c-half k%2 (2048 rows x 4096 points).
Host un-permutes output columns.
"""

import numpy as np

B, C, D, CH = 4, 4096, 4, 256
NCORES = 8
CW = C // 2            # rows per core
NBLK = CW // 128       # 16 row blocks
GW = 1024              # drain group width (points)
NG = C // GW           # 4 groups per row block
NF = 81                # monomial features
HB = NBLK // 2         # blocks per epilogue half
HC = HB * 128          # columns per epilogue half

MODE = "fp32"          # "fp32" | "f32r" (matmul input dtype for 81-feat path)

_cache = {}


def _build(bench_nrep=None, bench_span="full", mode=None):
    import concourse.bacc as bacc
    import concourse.mybir as mybir
    from concourse.tile import TileContext

    if mode is None:
        mode = MODE
    f32 = mybir.dt.float32
    mmdt = {"fp32": mybir.dt.float32, "f32r": mybir.dt.float32r}[mode]
    Alu = mybir.AluOpType
    Act = mybir.ActivationFunctionType

    nc = bacc.Bacc(None, target_bir_lowering=False)
    ft_r = nc.declare_dram_parameter("ft", [NF, C], mmdt, isOutput=False)
    gt_r = nc.declare_dram_parameter("gt", [NF, CW], mmdt, isOutput=False)
    f01_r = nc.declare_dram_parameter("f01", [9, C], f32, isOutput=False)
    f23_r = nc.declare_dram_parameter("f23", [9, C], f32, isOutput=False)
    g01_r = nc.declare_dram_parameter("g01", [9, 128], f32, isOutput=False)
    g23_r = nc.declare_dram_parameter("g23", [9, 128], f32, isOutput=False)
    ps2_r = nc.declare_dram_parameter("ps2", [128, NBLK], f32, isOutput=False)
    xcol_r = nc.declare_dram_parameter("xcol", [128, 2], f32, isOutput=False)
    y = nc.declare_dram_parameter("y", [CH, CW], f32, isOutput=True)

    def recip_accum_act(eng, in_ap, junk_ap, accum_ap):
        """ACT Reciprocal with free-dim row-sum accumulate (raw emit: the
        bass wrapper refuses Reciprocal; its table is ~1.2e-5 max rel err,
        fine for summing positive terms)."""
        imm = lambda v: mybir.ImmediateValue(dtype=mybir.dt.float32, value=v)
        eng.add_instruction(
            mybir.InstActivation(
                name=nc.get_next_instruction_name(),
                func=Act.Reciprocal,
                ins=[eng.lower_ap(in_ap), imm(0.0), imm(1.0), imm(0.0)],
                outs=[eng.lower_ap(junk_ap), eng.lower_ap(accum_ap)],
            )
        )

    with TileContext(nc) as tc:
        with (
            tc.tile_pool(name="persist", bufs=1) as pp,
            tc.tile_pool(name="work", bufs=1) as wp,
            tc.tile_pool(name="psum", bufs=1, space="PSUM") as psp,
            tc.tile_pool(name="dram", bufs=1, space="DRAM") as dp,
        ):
            scr = dp.tile([CW], f32, name="scr")
            ft = pp.tile([NF, C], mmdt)
            gt = pp.tile([NF, CW], mmdt)
            f01 = pp.tile([9, C], f32)
            f23 = pp.tile([9, C], f32)
            g01 = pp.tile([9, 128], f32)
            g23 = pp.tile([9, 128], f32)
            ps2_sb = pp.tile([128, NBLK], f32)
            xcol = pp.tile([128, 2], f32)
            Racc = pp.tile([128, NBLK, NG], f32)
            Racc0 = pp.tile([128, C // 512], f32)

            def loads():
                for q in range(NG):
                    nc.sync.dma_start(
                        out=ft[:, q * GW : (q + 1) * GW],
                        in_=ft_r[:, q * GW : (q + 1) * GW],
                    )
                for q in range(4):
                    w = CW // 4
                    nc.sync.dma_start(
                        out=gt[:, q * w : (q + 1) * w],
                        in_=gt_r[:, q * w : (q + 1) * w],
                    )
                nc.sync.dma_start(out=f01[:, :], in_=f01_r[:, :])
                nc.sync.dma_start(out=f23[:, :], in_=f23_r[:, :])
                nc.sync.dma_start(out=g01[:, :], in_=g01_r[:, :])
                nc.sync.dma_start(out=g23[:, :], in_=g23_r[:, :])
                nc.sync.dma_start(out=ps2_sb[:, :], in_=ps2_r[:, :])
                nc.sync.dma_start(out=xcol[:, :], in_=xcol_r[:, :])
                nc.vector.memset(Racc[:, 0, :], 0.0)

            def block0():
                # rows 0-127 (smallest s2prod): Q = (G01.F01)*(G23.F23)
                for j in range(C // 512):
                    sl = slice(j * 512, (j + 1) * 512)
                    qt = psp.tile([128, GW], f32, tag="q", bufs=4, name="qp")
                    nc.tensor.matmul(
                        qt[:, 0:512], g01[:, :], f01[:, sl], start=True, stop=True
                    )
                    nc.tensor.matmul(
                        qt[:, 512:GW], g23[:, :], f23[:, sl], start=True, stop=True
                    )
                    rA = wp.tile([128, 512], f32, tag="rA", bufs=2, name="rA")
                    junkA = wp.tile([128, 512], f32, tag="jA", bufs=2, name="jA")
                    recip_accum_act(nc.scalar, qt[:, 0:512], rA[:, :], junkA[:, 0:1])
                    rB = wp.tile([128, 512], f32, tag="rB", bufs=2, name="rB")
                    nc.vector.reciprocal_approx_fast(out=rB[:, :], in_=qt[:, 512:GW])
                    junkB = wp.tile([128, 512], f32, tag="jB", bufs=2, name="jB")
                    nc.vector.affine_mul_reduce(
                        out=junkB[:, :], accum_out=Racc0[:, j : j + 1],
                        in0=rA[:, :], in1=rB[:, :], scale=1.0, bias=0.0,
                    )

            def main_blocks(n_lo, n_hi):
                for n in range(n_lo, n_hi):
                    for g in range(NG):
                        qt = psp.tile([128, GW], f32, tag="q", bufs=4, name="qt")
                        for c2 in range(GW // 512):
                            sl = slice(g * GW + c2 * 512, g * GW + (c2 + 1) * 512)
                            nc.tensor.matmul(
                                qt[:, c2 * 512 : (c2 + 1) * 512],
                                gt[:, n * 128 : (n + 1) * 128],
                                ft[:, sl],
                                start=True,
                                stop=True,
                            )
                        if (n * NG + g) % 3 == 2:
                            r = wp.tile([128, GW], f32, tag="r", bufs=2, name="r")
                            nc.vector.reciprocal_approx_fast(out=r[:, :], in_=qt[:, :])
                            junkD = wp.tile([128, GW], f32, tag="jD", bufs=2, name="jD")
                            nc.vector.tensor_scalar(
                                junkD[:, :], r[:, :], 0.0, None, Alu.add, Alu.add,
                                accum_out=Racc[:, n, g : g + 1],
                            )
                        else:
                            junkC = wp.tile([128, GW], f32, tag="jC", bufs=2, name="jC")
                            recip_accum_act(
                                nc.scalar, qt[:, :], junkC[:, :], Racc[:, n, g : g + 1]
                            )

            def epilogue(half):
                nsl = slice(half * HB, (half + 1) * HB)
                t1 = wp.tile([128, HB], f32, tag="t1", bufs=2, name="t1")
                nc.vector.tensor_tensor(
                    t1[:, :], Racc[:, nsl, 0], Racc[:, nsl, 1], Alu.add
                )
                t2 = wp.tile([128, HB], f32, tag="t2", bufs=2, name="t2")
                nc.vector.tensor_tensor(
                    t2[:, :], Racc[:, nsl, 2], Racc[:, nsl, 3], Alu.add
                )
                Rsum = wp.tile([128, HB], f32, tag="Rs", bufs=2, name="Rs")
                nc.vector.tensor_tensor(Rsum[:, :], t1[:, :], t2[:, :], Alu.add)
                if half == 0:
                    jr = wp.tile([128, C // 512], f32, tag="jr", bufs=2, name="jr")
                    nc.vector.tensor_scalar(
                        jr[:, :], Racc0[:, :], 0.0, None, Alu.add, Alu.add,
                        accum_out=Rsum[:, 0:1],
                    )
                S = wp.tile([128, HB], f32, tag="S", bufs=2, name="S")
                nc.vector.tensor_tensor(S[:, :], Rsum[:, :], ps2_sb[:, nsl], Alu.mult)
                nc.vector.tensor_scalar_max(S[:, :], S[:, :], 1.0)
                coef = wp.tile([128, HB], f32, tag="coef", bufs=2, name="coef")
                nc.vector.reciprocal(coef[:, :], S[:, :])

                # transpose (128, HB) -> c-ordered row via DRAM bounce
                nc.sync.dma_start(
                    out=scr.rearrange("(n p) -> p n", p=128)[:, nsl], in_=coef[:, :]
                )
                cbc = wp.tile([128, HC], f32, tag="cbc", bufs=2, name="cbc")
                nc.sync.dma_start(
                    out=cbc[:, :],
                    in_=scr.rearrange("(one c) -> one c", one=1)[
                        0:1, half * HC : (half + 1) * HC
                    ].broadcast_to([128, HC]),
                )
                for h in range(CH // 128):
                    zt = wp.tile([128, HC], f32, tag="zt", bufs=2, name="zt")
                    nc.vector.tensor_scalar_mul(zt[:, :], cbc[:, :], xcol[:, h : h + 1])
                    nc.sync.dma_start(
                        out=y[h * 128 : (h + 1) * 128, half * HC : (half + 1) * HC],
                        in_=zt[:, :],
                    )

            def whole():
                loads()
                block0()
                main_blocks(1, HB)
                epilogue(0)
                main_blocks(HB, NBLK)
                epilogue(1)

            if bench_nrep is None:
                whole()
            else:
                import concourse.mybir as _mb

                with tc.For_i(
                    0, bench_nrep, 1,
                    staggered_reset=True,
                    hint_engines=(_mb.EngineType.DVE, _mb.EngineType.Activation),
                ):
                    whole()
    nc.finalize()
    return nc


def _get_nc():
    if "nc" not in _cache:
        _cache["nc"] = _build()
    return _cache["nc"]


_IDX4 = np.indices((3, 3, 3, 3)).reshape(4, -1).T  # (81, 4) exponent tuples
_IDX2 = np.indices((3, 3)).reshape(2, -1).T        # (9, 2)


def _feat(m, s2, dims, idx):
    """G (rows, nf), F (points, nf) in float64 for the given dims."""
    n = m.shape[0]
    G = np.ones((n, len(idx)))
    F = np.ones((n, len(idx)))
    for e, exps in enumerate(idx):
        for d, ed in zip(dims, exps):
            gd = [s2[:, d] + m[:, d] ** 2, -2.0 * m[:, d], np.ones(n)][ed]
            fd = [np.ones(n), m[:, d], m[:, d] ** 2][ed]
            G[:, e] = G[:, e] * gd
            F[:, e] = F[:, e] * fd
    return G, F


def _in_maps(x, mu, sig):
    maps = []
    perms = []
    for k in range(NCORES):
        b, half = k // 2, k % 2
        m = mu[b].astype(np.float64) - 0.5        # centered, (C, D)
        s2 = sig[b].astype(np.float64) ** 2
        s2p = s2.prod(axis=1)
        order = np.argsort(
            s2p[half * CW : (half + 1) * CW], kind="stable"
        )  # within-half ascending s2prod
        rows = half * CW + order
        G, F = _feat(m, s2, (0, 1, 2, 3), _IDX4)
        G01, F01 = _feat(m, s2, (0, 1), _IDX2)
        G23, F23 = _feat(m, s2, (2, 3), _IDX2)
        r0 = rows[:128]
        maps.append(
            {
                "ft": np.ascontiguousarray(F.T, np.float32),
                "gt": np.ascontiguousarray(G[rows].T, np.float32),
                "f01": np.ascontiguousarray(F01.T, np.float32),
                "f23": np.ascontiguousarray(F23.T, np.float32),
                "g01": np.ascontiguousarray(G01[r0].T, np.float32),
                "g23": np.ascontiguousarray(G23[r0].T, np.float32),
                "ps2": np.ascontiguousarray(
                    s2p[rows].reshape(NBLK, 128).T, np.float32
                ),
                "xcol": np.ascontiguousarray(
                    x[b, :, 0].reshape(2, 128).T, np.float32
                ),
            }
        )
        perms.append(order)
    return maps, perms


def kernel(x, pi, mu, sig):
    from concourse.bass_utils import run_bass_kernel_spmd

    nc = _get_nc()
    maps, perms = _in_maps(x, mu, sig)
    res = run_bass_kernel_spmd(nc, maps, list(range(NCORES))).results
    y = np.empty((B, CH, C), np.float32)
    for k in range(NCORES):
        b, half = k // 2, k % 2
        y[b][:, half * CW + perms[k]] = res[k]["y"]
    return y


# revision 4
# speedup vs baseline: 1.4653x; 1.0190x over previous
"""Trainium2 Bass kernel for nn_MixtureAttention.

Math: the reference builds a (c,c) pairwise Cauchy-product matrix per batch,
row-normalizes it, and keeps only the diagonal.  `pi` cancels; with
    Q[i,p] = prod_d (sig[i,d]^2 + (mu[p,d]-mu[i,d])^2)
    S[i]   = s2prod[i] * sum_p 1/Q[i,p]        (s2prod = prod_d sig^2)
the kept diagonal is coef[i] = 1/S[i], and y[b,ch,c] = x[b,ch]*coef[b,c].

Key idea: Q[i,p] is a polynomial in mu[p,:]; with centered coords m'=mu-0.5
it separates into an 81-monomial feature contraction Q = G[i,:].F[p,:] on
the TensorEngine.  HW fp32 matmul is ~2.05us/MM while float32r (11-bit
mantissa, RNE — measured) runs ~0.1us/MM, so each operand is split
hi/lo at 11 mantissa bits and Q is accumulated in 3 f32r passes
(hh, hl, lh) — ~fp32 accuracy at f32r speed.  The drain is a fused
Reciprocal+row-sum pass (ACT, with DVE taking every 3rd group).

Numerics: rows are host-sorted by s2prod ascending; each core's first
128-row block (tiny s2prod = ill-conditioned) instead uses the 2+2 split
Q = (G01.F01)*(G23.F23) (9 features, same 3-pass f32r), and S is floored
at 1.  Validated vs fp64: metric ~1.5e-4 (gate 2e-2).

Sharding: core k handles batch k//2, c-half k%2 (2048 rows x 4096 points).
Host un-permutes output columns.
"""

import numpy as np

B, C, D, CH = 4, 4096, 4, 256
NCORES = 8
CW = C // 2            # rows per core
NBLK = CW // 128       # 16 row blocks
GW = 1024              # drain group width (points)
NG = C // GW           # 4 groups per row block
NF = 81                # monomial features
HB = NBLK // 2         # blocks per epilogue half
HC = HB * 128          # columns per epilogue half

_cache = {}


def _build(bench_nrep=None, bench_span="full"):
    import concourse.bacc as bacc
    import concourse.mybir as mybir
    from concourse.tile import TileContext

    f32 = mybir.dt.float32
    fr = mybir.dt.float32r
    Alu = mybir.AluOpType
    Act = mybir.ActivationFunctionType

    nc = bacc.Bacc(None, target_bir_lowering=False)
    fth_r = nc.declare_dram_parameter("fth", [NF, C], fr, isOutput=False)
    ftl_r = nc.declare_dram_parameter("ftl", [NF, C], fr, isOutput=False)
    fph_r = nc.declare_dram_parameter("fph", [9, 2 * C], fr, isOutput=False)
    fpl_r = nc.declare_dram_parameter("fpl", [9, 2 * C], fr, isOutput=False)
    gth_r = nc.declare_dram_parameter("gth", [NF, CW], fr, isOutput=False)
    gtl_r = nc.declare_dram_parameter("gtl", [NF, CW], fr, isOutput=False)
    gph_r = nc.declare_dram_parameter("gph", [9, 256], fr, isOutput=False)
    gpl_r = nc.declare_dram_parameter("gpl", [9, 256], fr, isOutput=False)
    ps2_r = nc.declare_dram_parameter("ps2", [128, NBLK], f32, isOutput=False)
    xcol_r = nc.declare_dram_parameter("xcol", [128, 2], f32, isOutput=False)
    y = nc.declare_dram_parameter("y", [CH, CW], f32, isOutput=True)

    def recip_accum_act(eng, in_ap, junk_ap, accum_ap):
        """ACT Reciprocal with free-dim row-sum accumulate (raw emit: the
        bass wrapper refuses Reciprocal; its table is ~1.2e-5 max rel err,
        fine for summing positive terms)."""
        imm = lambda v: mybir.ImmediateValue(dtype=mybir.dt.float32, value=v)
        eng.add_instruction(
            mybir.InstActivation(
                name=nc.get_next_instruction_name(),
                func=Act.Reciprocal,
                ins=[eng.lower_ap(in_ap), imm(0.0), imm(1.0), imm(0.0)],
                outs=[eng.lower_ap(junk_ap), eng.lower_ap(accum_ap)],
            )
        )

    with TileContext(nc) as tc:
        with (
            tc.tile_pool(name="persist", bufs=1) as pp,
            tc.tile_pool(name="work", bufs=1) as wp,
            tc.tile_pool(name="psum", bufs=1, space="PSUM") as psp,
            tc.tile_pool(name="dram", bufs=1, space="DRAM") as dp,
        ):
            scr = dp.tile([CW], f32, name="scr")
            fth = pp.tile([NF, C], fr)
            ftl = pp.tile([NF, C], fr)
            fph = pp.tile([9, 2 * C], fr)
            fpl = pp.tile([9, 2 * C], fr)
            gth = pp.tile([NF, CW], fr)
            gtl = pp.tile([NF, CW], fr)
            gph = pp.tile([9, 256], fr)
            gpl = pp.tile([9, 256], fr)
            ps2_sb = pp.tile([128, NBLK], f32)
            xcol = pp.tile([128, 2], f32)
            Racc = pp.tile([128, NBLK, NG], f32)
            Racc0 = pp.tile([128, C // 512], f32)

            def loads():
                for q in range(NG):
                    sl = slice(q * GW, (q + 1) * GW)
                    nc.sync.dma_start(out=fth[:, sl], in_=fth_r[:, sl])
                    nc.sync.dma_start(out=ftl[:, sl], in_=ftl_r[:, sl])
                nc.sync.dma_start(out=fph[:, :], in_=fph_r[:, :])
                nc.sync.dma_start(out=fpl[:, :], in_=fpl_r[:, :])
                for q in range(4):
                    w = CW // 4
                    sl = slice(q * w, (q + 1) * w)
                    nc.sync.dma_start(out=gth[:, sl], in_=gth_r[:, sl])
                    nc.sync.dma_start(out=gtl[:, sl], in_=gtl_r[:, sl])
                nc.sync.dma_start(out=gph[:, :], in_=gph_r[:, :])
                nc.sync.dma_start(out=gpl[:, :], in_=gpl_r[:, :])
                nc.sync.dma_start(out=ps2_sb[:, :], in_=ps2_r[:, :])
                nc.sync.dma_start(out=xcol[:, :], in_=xcol_r[:, :])
                nc.vector.memset(Racc[:, 0, :], 0.0)

            def mm3(out_ap, gh, gl, fh, fl, gsl, fsl):
                nc.tensor.matmul(out_ap, gh[:, gsl], fh[:, fsl], start=True, stop=False)
                nc.tensor.matmul(out_ap, gh[:, gsl], fl[:, fsl], start=False, stop=False)
                nc.tensor.matmul(out_ap, gl[:, gsl], fh[:, fsl], start=False, stop=True)

            def block0():
                # rows 0-127 (smallest s2prod): Q = (G01.F01)*(G23.F23)
                for j in range(C // 512):
                    sl = slice(j * 512, (j + 1) * 512)
                    sl2 = slice(C + j * 512, C + (j + 1) * 512)
                    qt = psp.tile([128, GW], f32, tag="q", bufs=4, name="qp")
                    mm3(qt[:, 0:512], gph, gpl, fph, fpl, slice(0, 128), sl)
                    mm3(qt[:, 512:GW], gph, gpl, fph, fpl, slice(128, 256), sl2)
                    rA = wp.tile([128, 512], f32, tag="rA", bufs=2, name="rA")
                    junkA = wp.tile([128, 512], f32, tag="jA", bufs=2, name="jA")
                    recip_accum_act(nc.scalar, qt[:, 0:512], rA[:, :], junkA[:, 0:1])
                    rB = wp.tile([128, 512], f32, tag="rB", bufs=2, name="rB")
                    nc.vector.reciprocal_approx_fast(out=rB[:, :], in_=qt[:, 512:GW])
                    junkB = wp.tile([128, 512], f32, tag="jB", bufs=2, name="jB")
                    nc.vector.affine_mul_reduce(
                        out=junkB[:, :], accum_out=Racc0[:, j : j + 1],
                        in0=rA[:, :], in1=rB[:, :], scale=1.0, bias=0.0,
                    )

            def main_blocks(n_lo, n_hi):
                for n in range(n_lo, n_hi):
                    gsl = slice(n * 128, (n + 1) * 128)
                    for g in range(NG):
                        qt = psp.tile([128, GW], f32, tag="q", bufs=4, name="qt")
                        for c2 in range(GW // 512):
                            sl = slice(g * GW + c2 * 512, g * GW + (c2 + 1) * 512)
                            mm3(
                                qt[:, c2 * 512 : (c2 + 1) * 512],
                                gth, gtl, fth, ftl, gsl, sl,
                            )
                        if (n * NG + g) % 3 == 2:
                            r = wp.tile([128, GW], f32, tag="r", bufs=2, name="r")
                            nc.vector.reciprocal_approx_fast(out=r[:, :], in_=qt[:, :])
                            junkD = wp.tile([128, GW], f32, tag="jD", bufs=2, name="jD")
                            nc.vector.tensor_scalar(
                                junkD[:, :], r[:, :], 0.0, None, Alu.add, Alu.add,
                                accum_out=Racc[:, n, g : g + 1],
                            )
                        else:
                            junkC = wp.tile([128, GW], f32, tag="jC", bufs=2, name="jC")
                            recip_accum_act(
                                nc.scalar, qt[:, :], junkC[:, :], Racc[:, n, g : g + 1]
                            )

            def epilogue(half):
                nsl = slice(half * HB, (half + 1) * HB)
                t1 = wp.tile([128, HB], f32, tag="t1", bufs=2, name="t1")
                nc.vector.tensor_tensor(
                    t1[:, :], Racc[:, nsl, 0], Racc[:, nsl, 1], Alu.add
                )
                t2 = wp.tile([128, HB], f32, tag="t2", bufs=2, name="t2")
                nc.vector.tensor_tensor(
                    t2[:, :], Racc[:, nsl, 2], Racc[:, nsl, 3], Alu.add
                )
                Rsum = wp.tile([128, HB], f32, tag="Rs", bufs=2, name="Rs")
                nc.vector.tensor_tensor(Rsum[:, :], t1[:, :], t2[:, :], Alu.add)
                if half == 0:
                    jr = wp.tile([128, C // 512], f32, tag="jr", bufs=2, name="jr")
                    nc.vector.tensor_scalar(
                        jr[:, :], Racc0[:, :], 0.0, None, Alu.add, Alu.add,
                        accum_out=Rsum[:, 0:1],
                    )
                S = wp.tile([128, HB], f32, tag="S", bufs=2, name="S")
                nc.vector.tensor_tensor(S[:, :], Rsum[:, :], ps2_sb[:, nsl], Alu.mult)
                nc.vector.tensor_scalar_max(S[:, :], S[:, :], 1.0)
                coef = wp.tile([128, HB], f32, tag="coef", bufs=2, name="coef")
                nc.vector.reciprocal(coef[:, :], S[:, :])

                # transpose (128, HB) -> c-ordered row via DRAM bounce
                nc.sync.dma_start(
                    out=scr.rearrange("(n p) -> p n", p=128)[:, nsl], in_=coef[:, :]
                )
                cbc = wp.tile([128, HC], f32, tag="cbc", bufs=2, name="cbc")
                nc.sync.dma_start(
                    out=cbc[:, :],
                    in_=scr.rearrange("(one c) -> one c", one=1)[
                        0:1, half * HC : (half + 1) * HC
                    ].broadcast_to([128, HC]),
                )
                for h in range(CH // 128):
                    zt = wp.tile([128, HC], f32, tag="zt", bufs=2, name="zt")
                    nc.vector.tensor_scalar_mul(zt[:, :], cbc[:, :], xcol[:, h : h + 1])
                    nc.sync.dma_start(
                        out=y[h * 128 : (h + 1) * 128, half * HC : (half + 1) * HC],
                        in_=zt[:, :],
                    )

            def whole():
                loads()
                block0()
                main_blocks(1, HB)
                epilogue(0)
                main_blocks(HB, NBLK)
                epilogue(1)

            if bench_nrep is None:
                whole()
            else:
                import concourse.mybir as _mb

                with tc.For_i(
                    0, bench_nrep, 1,
                    staggered_reset=True,
                    hint_engines=(_mb.EngineType.DVE, _mb.EngineType.Activation),
                ):
                    whole()
    nc.finalize()
    return nc


def _get_nc():
    if "nc" not in _cache:
        _cache["nc"] = _build()
    return _cache["nc"]


_IDX4 = np.indices((3, 3, 3, 3)).reshape(4, -1).T  # (81, 4) exponent tuples
_IDX2 = np.indices((3, 3)).reshape(2, -1).T        # (9, 2)


def _feat(m, s2, dims, idx):
    """G (rows, nf), F (points, nf) in float64 for the given dims."""
    n = m.shape[0]
    G = np.ones((n, len(idx)))
    F = np.ones((n, len(idx)))
    for e, exps in enumerate(idx):
        for d, ed in zip(dims, exps):
            gd = [s2[:, d] + m[:, d] ** 2, -2.0 * m[:, d], np.ones(n)][ed]
            fd = [np.ones(n), m[:, d], m[:, d] ** 2][ed]
            G[:, e] = G[:, e] * gd
            F[:, e] = F[:, e] * fd
    return G, F


def _round11(a):
    """Round fp32 array to 11 mantissa bits (RNE) — f32r's storage grid."""
    bits = np.asarray(a, np.float32).view(np.uint32).astype(np.uint64)
    sh = np.uint64(12)  # 23 - 11
    add = (np.uint64(1) << np.uint64(11)) - np.uint64(1) + ((bits >> sh) & np.uint64(1))
    return (((bits + add) >> sh) << sh).astype(np.uint32).view(np.float32)


def _split11(a):
    h = _round11(np.asarray(a, np.float32))
    l = _round11((np.asarray(a, np.float32) - h).astype(np.float32))
    return h, l


def _in_maps(x, mu, sig):
    maps = []
    perms = []
    for k in range(NCORES):
        b, half = k // 2, k % 2
        m = mu[b].astype(np.float64) - 0.5        # centered, (C, D)
        s2 = sig[b].astype(np.float64) ** 2
        s2p = s2.prod(axis=1)
        order = np.argsort(
            s2p[half * CW : (half + 1) * CW], kind="stable"
        )  # within-half ascending s2prod
        rows = half * CW + order
        G, F = _feat(m, s2, (0, 1, 2, 3), _IDX4)
        G01, F01 = _feat(m, s2, (0, 1), _IDX2)
        G23, F23 = _feat(m, s2, (2, 3), _IDX2)
        r0 = rows[:128]
        fth, ftl = _split11(F.T)
        gth, gtl = _split11(G[rows].T)
        fph, fpl = _split11(np.concatenate([F01.T, F23.T], axis=1))
        gph, gpl = _split11(np.concatenate([G01[r0].T, G23[r0].T], axis=1))
        maps.append(
            {
                "fth": np.ascontiguousarray(fth),
                "ftl": np.ascontiguousarray(ftl),
                "fph": np.ascontiguousarray(fph),
                "fpl": np.ascontiguousarray(fpl),
                "gth": np.ascontiguousarray(gth),
                "gtl": np.ascontiguousarray(gtl),
                "gph": np.ascontiguousarray(gph),
                "gpl": np.ascontiguousarray(gpl),
                "ps2": np.ascontiguousarray(
                    s2p[rows].reshape(NBLK, 128).T, np.float32
                ),
                "xcol": np.ascontiguousarray(
                    x[b, :, 0].reshape(2, 128).T, np.float32
                ),
            }
        )
        perms.append(order)
    return maps, perms


def kernel(x, pi, mu, sig):
    from concourse.bass_utils import run_bass_kernel_spmd

    nc = _get_nc()
    maps, perms = _in_maps(x, mu, sig)
    res = run_bass_kernel_spmd(nc, maps, list(range(NCORES))).results
    y = np.empty((B, CH, C), np.float32)
    for k in range(NCORES):
        b, half = k // 2, k % 2
        y[b][:, half * CW + perms[k]] = res[k]["y"]
    return y


# revision 7
# speedup vs baseline: 2.1735x; 1.4833x over previous
"""Trainium2 Bass kernel for nn_MixtureAttention.

Math: the reference builds a (c,c) pairwise Cauchy-product matrix per batch,
row-normalizes it, and keeps only the diagonal.  `pi` cancels; with
    Q[i,p] = prod_d (sig[i,d]^2 + (mu[p,d]-mu[i,d])^2)
    S[i]   = s2prod[i] * sum_p 1/Q[i,p]        (s2prod = prod_d sig^2)
the kept diagonal is coef[i] = 1/S[i], and y[b,ch,c] = x[b,ch]*coef[b,c].

Key idea: Q[i,p] is a polynomial in mu[p,:]; with centered coords m'=mu-0.5
it separates into an 81-monomial feature contraction Q = G[i,:].F[p,:] on
the TensorEngine.  fp32 matmul measures ~2.05us/MM on HW, so instead each
operand is split 3-way in bf16 (8+8+8 mantissa bits) and the 6 cross-term
products with combined split-level <= 2 are kept — 486 K-rows, zero-padded
to 4 K=128 parts, accumulated into each PSUM chunk.  That's ~fp32 accuracy
at bf16 speed.  MMs are issued weight-major over half-blocks (one lhsT
part swept across 4 chunks before switching) — LDWEIGHTS reuse makes MMs
~2x faster than chunk-major order (116 vs 243 ns/MM measured).

The drain is a fused Reciprocal+row-sum pass (ACT Reciprocal via raw emit,
~1.2e-5 rel err; DVE takes a share via reciprocal_approx_fast+accum).

Numerics: rows are host-sorted by s2prod ascending; each core's first
128-row block (tiny s2prod = ill-conditioned) instead uses the 2+2 split
Q = (G01.F01)*(G23.F23) (9 features, 54 packed rows), and S is floored
at 1.  Validated vs fp64: metric ~1.4e-4 (gate 2e-2).

Sharding: core k handles batch k//2, c-half k%2 (2048 rows x 4096 points).
Host un-permutes output columns.
"""

import numpy as np

B, C, D, CH = 4, 4096, 4, 256
NCORES = 8
CW = C // 2            # rows per core
NBLK = CW // 128       # 16 row blocks
NP = 4                 # packed K-parts (486 rows -> 4 x 128)
HB = NBLK // 2         # blocks per epilogue half
HC = HB * 128          # columns per epilogue half

_cache = {}


def _build(bench_nrep=None, bench_span="full", parts=None):
    import concourse.bacc as bacc
    import concourse.mybir as mybir
    from concourse.tile import TileContext

    if parts is None:
        parts = {"block0", "main", "drain", "epi"}

    f32 = mybir.dt.float32
    bf = mybir.dt.bfloat16
    Alu = mybir.AluOpType
    Act = mybir.ActivationFunctionType

    nc = bacc.Bacc(None, target_bir_lowering=False)
    fp_r = nc.declare_dram_parameter("fp", [128, NP * C], bf, isOutput=False)
    gp_r = nc.declare_dram_parameter("gp", [128, NP * CW], bf, isOutput=False)
    fp0_r = nc.declare_dram_parameter("fp0", [54, 2 * C], bf, isOutput=False)
    gp0_r = nc.declare_dram_parameter("gp0", [54, 256], bf, isOutput=False)
    ps2_r = nc.declare_dram_parameter("ps2", [128, NBLK], f32, isOutput=False)
    xcol_r = nc.declare_dram_parameter("xcol", [128, 2], f32, isOutput=False)
    y = nc.declare_dram_parameter("y", [CH, CW], f32, isOutput=True)

    def recip_accum_act(in_ap, junk_ap, accum_ap):
        """ACT Reciprocal with free-dim row-sum accumulate (raw emit: the
        bass wrapper refuses Reciprocal; its table is ~1.2e-5 max rel err,
        fine for summing positive terms)."""
        eng = nc.scalar
        imm = lambda v: mybir.ImmediateValue(dtype=mybir.dt.float32, value=v)
        eng.add_instruction(
            mybir.InstActivation(
                name=nc.get_next_instruction_name(),
                func=Act.Reciprocal,
                ins=[eng.lower_ap(in_ap), imm(0.0), imm(1.0), imm(0.0)],
                outs=[eng.lower_ap(junk_ap), eng.lower_ap(accum_ap)],
            )
        )

    with TileContext(nc) as tc:
        with (
            tc.tile_pool(name="persist", bufs=1) as pp,
            tc.tile_pool(name="work", bufs=1) as wp,
            tc.tile_pool(name="psum", bufs=1, space="PSUM") as psp,
            tc.tile_pool(name="dram", bufs=1, space="DRAM") as dp,
        ):
            scr = dp.tile([CW], f32, name="scr")
            fp = pp.tile([128, NP * C], bf)
            gp = pp.tile([128, NP * CW], bf)
            fp0 = pp.tile([54, 2 * C], bf)
            gp0 = pp.tile([54, 256], bf)
            ps2_sb = pp.tile([128, NBLK], f32)
            xcol = pp.tile([128, 2], f32)
            Racc = pp.tile([128, NBLK, 4], f32)
            Racc0 = pp.tile([128, C // 512], f32)

            def loads():
                for q in range(NP):
                    nc.sync.dma_start(
                        out=fp[:, q * C : (q + 1) * C],
                        in_=fp_r[:, q * C : (q + 1) * C],
                    )
                for q in range(2):
                    w = NP * CW // 2
                    nc.sync.dma_start(
                        out=gp[:, q * w : (q + 1) * w],
                        in_=gp_r[:, q * w : (q + 1) * w],
                    )
                nc.sync.dma_start(out=fp0[:, :], in_=fp0_r[:, :])
                nc.sync.dma_start(out=gp0[:, :], in_=gp0_r[:, :])
                nc.sync.dma_start(out=ps2_sb[:, :], in_=ps2_r[:, :])
                nc.sync.dma_start(out=xcol[:, :], in_=xcol_r[:, :])
                nc.vector.memset(Racc[:, 0, :], 0.0)

            def drain_main(qt, n, g):
                # qt: (128, 1024) PSUM group = points [g*1024, (g+1)*1024)
                if (n * 2 + g) % 8 < 5:
                    junkC = wp.tile([128, 1024], f32, tag="jC", bufs=2, name="jC")
                    recip_accum_act(qt[:, :], junkC[:, :], Racc[:, n, g : g + 1])
                else:
                    r = wp.tile([128, 1024], f32, tag="r", bufs=2, name="r")
                    nc.vector.reciprocal_approx_fast(out=r[:, :], in_=qt[:, :])
                    junkD = wp.tile([128, 1024], f32, tag="jD", bufs=2, name="jD")
                    nc.vector.tensor_scalar(
                        junkD[:, :], r[:, :], 0.0, None, Alu.add, Alu.add,
                        accum_out=Racc[:, n, g : g + 1],
                    )

            def block0():
                # rows 0-127 (smallest s2prod): Q = (G01.F01)*(G23.F23)
                for j in range(C // 512):
                    sl = slice(j * 512, (j + 1) * 512)
                    sl2 = slice(C + j * 512, C + (j + 1) * 512)
                    qt = psp.tile([128, 1024], f32, tag="q", bufs=4, name="qp")
                    nc.tensor.matmul(
                        qt[:, 0:512], gp0[:, 0:128], fp0[:, sl],
                        start=True, stop=True,
                    )
                    nc.tensor.matmul(
                        qt[:, 512:1024], gp0[:, 128:256], fp0[:, sl2],
                        start=True, stop=True,
                    )
                    if "drain" in parts:
                        rA = wp.tile([128, 512], f32, tag="rA", bufs=2, name="rA")
                        junkA = wp.tile([128, 512], f32, tag="jA", bufs=2, name="jA")
                        recip_accum_act(qt[:, 0:512], rA[:, :], junkA[:, 0:1])
                        rB = wp.tile([128, 512], f32, tag="rB", bufs=2, name="rB")
                        nc.vector.reciprocal_approx_fast(
                            out=rB[:, :], in_=qt[:, 512:1024]
                        )
                        junkB = wp.tile([128, 512], f32, tag="jB", bufs=2, name="jB")
                        nc.vector.affine_mul_reduce(
                            out=junkB[:, :], accum_out=Racc0[:, j : j + 1],
                            in0=rA[:, :], in1=rB[:, :], scale=1.0, bias=0.0,
                        )

            def main_blocks(n_lo, n_hi):
                for n in range(n_lo, n_hi):
                    for hb in range(2):
                        # half-block: 4 chunks of 512 points = 2 PSUM tiles
                        pss = [
                            psp.tile([128, 1024], f32, tag="q", bufs=4, name="qt")
                            for _ in range(2)
                        ]
                        for q in range(NP):
                            gsl = slice(q * CW + n * 128, q * CW + (n + 1) * 128)
                            for j in range(4):
                                j0 = hb * 4 + j
                                nc.tensor.matmul(
                                    pss[j // 2][:, (j % 2) * 512 : (j % 2 + 1) * 512],
                                    gp[:, gsl],
                                    fp[:, q * C + j0 * 512 : q * C + (j0 + 1) * 512],
                                    start=(q == 0),
                                    stop=(q == NP - 1),
                                )
                        if "drain" in parts:
                            drain_main(pss[0], n, hb * 2)
                            drain_main(pss[1], n, hb * 2 + 1)

            def epilogue(half):
                nsl = slice(half * HB, (half + 1) * HB)
                t1 = wp.tile([128, HB], f32, tag="t1", bufs=2, name="t1")
                nc.vector.tensor_tensor(
                    t1[:, :], Racc[:, nsl, 0], Racc[:, nsl, 1], Alu.add
                )
                t2 = wp.tile([128, HB], f32, tag="t2", bufs=2, name="t2")
                nc.vector.tensor_tensor(
                    t2[:, :], Racc[:, nsl, 2], Racc[:, nsl, 3], Alu.add
                )
                Rsum = wp.tile([128, HB], f32, tag="Rs", bufs=2, name="Rs")
                nc.vector.tensor_tensor(Rsum[:, :], t1[:, :], t2[:, :], Alu.add)
                if half == 0:
                    jr = wp.tile([128, C // 512], f32, tag="jr", bufs=2, name="jr")
                    nc.vector.tensor_scalar(
                        jr[:, :], Racc0[:, :], 0.0, None, Alu.add, Alu.add,
                        accum_out=Rsum[:, 0:1],
                    )
                S = wp.tile([128, HB], f32, tag="S", bufs=2, name="S")
                nc.vector.tensor_tensor(S[:, :], Rsum[:, :], ps2_sb[:, nsl], Alu.mult)
                nc.vector.tensor_scalar_max(S[:, :], S[:, :], 1.0)
                coef = wp.tile([128, HB], f32, tag="coef", bufs=2, name="coef")
                nc.vector.reciprocal(coef[:, :], S[:, :])

                # transpose (128, HB) -> c-ordered row via DRAM bounce
                nc.sync.dma_start(
                    out=scr.rearrange("(n p) -> p n", p=128)[:, nsl], in_=coef[:, :]
                )
                cbc = wp.tile([128, HC], f32, tag="cbc", bufs=2, name="cbc")
                nc.sync.dma_start(
                    out=cbc[:, :],
                    in_=scr.rearrange("(one c) -> one c", one=1)[
                        0:1, half * HC : (half + 1) * HC
                    ].broadcast_to([128, HC]),
                )
                for h in range(CH // 128):
                    zt = wp.tile([128, HC], f32, tag="zt", bufs=2, name="zt")
                    nc.vector.tensor_scalar_mul(zt[:, :], cbc[:, :], xcol[:, h : h + 1])
                    nc.sync.dma_start(
                        out=y[h * 128 : (h + 1) * 128, half * HC : (half + 1) * HC],
                        in_=zt[:, :],
                    )

            def whole():
                loads()
                if "block0" in parts:
                    block0()
                if "main" in parts:
                    main_blocks(1, HB)
                if "epi" in parts and "drain" in parts:
                    epilogue(0)
                if "main" in parts:
                    main_blocks(HB, NBLK)
                if "epi" in parts and "drain" in parts:
                    epilogue(1)

            if bench_nrep is None:
                whole()
            else:
                import concourse.mybir as _mb

                with tc.For_i(
                    0, bench_nrep, 1,
                    staggered_reset=True,
                    hint_engines=(_mb.EngineType.DVE, _mb.EngineType.Activation),
                ):
                    whole()
    nc.finalize()
    return nc


def _get_nc():
    if "nc" not in _cache:
        _cache["nc"] = _build()
    return _cache["nc"]


_IDX4 = np.indices((3, 3, 3, 3)).reshape(4, -1).T  # (81, 4) exponent tuples
_IDX2 = np.indices((3, 3)).reshape(2, -1).T        # (9, 2)
_COMBOS = [(0, 0), (0, 1), (1, 0), (0, 2), (1, 1), (2, 0)]  # split levels i+j<=2


def _feat(m, s2, dims, idx):
    """G (rows, nf), F (points, nf) in float64 for the given dims."""
    n = m.shape[0]
    G = np.ones((n, len(idx)))
    F = np.ones((n, len(idx)))
    for e, exps in enumerate(idx):
        for d, ed in zip(dims, exps):
            gd = [s2[:, d] + m[:, d] ** 2, -2.0 * m[:, d], np.ones(n)][ed]
            fd = [np.ones(n), m[:, d], m[:, d] ** 2][ed]
            G[:, e] = G[:, e] * gd
            F[:, e] = F[:, e] * fd
    return G, F


def _bf16(a):
    bits = np.asarray(a, np.float32).view(np.uint32)
    r = ((bits.astype(np.uint64) + 0x7FFF + ((bits >> 16) & 1)) >> 16) << 16
    return r.astype(np.uint32).view(np.float32)


def _split3(a):
    a = np.asarray(a, np.float32)
    h = _bf16(a)
    m = _bf16((a - h).astype(np.float32))
    l = _bf16((a - h - m).astype(np.float32))
    return h, m, l


def _pack6(G, F):
    """bf16 3-way split, 6 cross-terms: (rows, 6nf), (points, 6nf)."""
    Gs = _split3(G.astype(np.float32))
    Fs = _split3(F.astype(np.float32))
    Gp = np.concatenate([Gs[i] for i, j in _COMBOS], axis=1)
    Fp = np.concatenate([Fs[j] for i, j in _COMBOS], axis=1)
    return Gp, Fp


def _to_parts(a, nrows, width):
    """(K, width) -> zero-pad K to NP*128 -> (128, NP*width) bf16."""
    import ml_dtypes

    pad = np.zeros((NP * 128, width), np.float32)
    pad[:nrows] = a
    return np.ascontiguousarray(
        pad.reshape(NP, 128, width).transpose(1, 0, 2).reshape(128, NP * width)
    ).astype(ml_dtypes.bfloat16)


def _in_maps(x, mu, sig):
    import ml_dtypes

    maps = []
    perms = []
    for k in range(NCORES):
        b, half = k // 2, k % 2
        m = mu[b].astype(np.float64) - 0.5        # centered, (C, D)
        s2 = sig[b].astype(np.float64) ** 2
        s2p = s2.prod(axis=1)
        order = np.argsort(
            s2p[half * CW : (half + 1) * CW], kind="stable"
        )  # within-half ascending s2prod
        rows = half * CW + order
        G, F = _feat(m, s2, (0, 1, 2, 3), _IDX4)
        G01, F01 = _feat(m, s2, (0, 1), _IDX2)
        G23, F23 = _feat(m, s2, (2, 3), _IDX2)
        r0 = rows[:128]
        Gp, Fp = _pack6(G[rows], F)               # (2048, 486), (4096, 486)
        Gp0, Fp0 = _pack6(G01[r0], F01)           # (128, 54), (4096, 54)
        Gp2, Fp2 = _pack6(G23[r0], F23)
        maps.append(
            {
                "fp": _to_parts(Fp.T, 486, C),
                "gp": _to_parts(Gp.T, 486, CW),
                "fp0": np.ascontiguousarray(
                    np.concatenate([Fp0.T, Fp2.T], axis=1)
                ).astype(ml_dtypes.bfloat16),
                "gp0": np.ascontiguousarray(
                    np.concatenate([Gp0.T, Gp2.T], axis=1)
                ).astype(ml_dtypes.bfloat16),
                "ps2": np.ascontiguousarray(
                    s2p[rows].reshape(NBLK, 128).T, np.float32
                ),
                "xcol": np.ascontiguousarray(
                    x[b, :, 0].reshape(2, 128).T, np.float32
                ),
            }
        )
        perms.append(order)
    return maps, perms


def kernel(x, pi, mu, sig):
    from concourse.bass_utils import run_bass_kernel_spmd

    nc = _get_nc()
    maps, perms = _in_maps(x, mu, sig)
    res = run_bass_kernel_spmd(nc, maps, list(range(NCORES))).results
    y = np.empty((B, CH, C), np.float32)
    for k in range(NCORES):
        b, half = k // 2, k % 2
        y[b][:, half * CW + perms[k]] = res[k]["y"]
    return y
